# revision 3
# baseline (speedup 1.0000x reference)
"""MoE FFN (16 experts, top-4, null-expert router) on 8 Trainium2 NeuronCores.

Strategy:
  - Router, top-k selection, combine weights and aux losses are computed on
    host in numpy (~0.15 GFLOP total; the device work is ~258 GFLOP).
  - Routed experts are expert-parallel: 2 experts per core (paired
    large+small count for balance), each expert's gathered tokens padded to
    a common capacity so all 8 cores run an identical program.
  - The shared expert is token-parallel: 512 tokens per core.
  - All GEMMs run on the tensor engine in bf16 with fp32 PSUM accumulation.
  - Host applies combine weights and scatter-adds expert outputs.
"""

import math

import ml_dtypes
import numpy as np

B, T, D, H = 2, 2048, 1024, 2048
E = 16
K = 4
RHO = 0.5
NULL = int(E * (1 - RHO) / RHO)
N = B * T
N_CORES = 8
BF16 = ml_dtypes.bfloat16


# ---------------------------------------------------------------------------
# Host-side router (exact replication of the reference math in numpy)
# ---------------------------------------------------------------------------

def _route(x, gate_w, logit_bias, null_logit):
    xf = np.ascontiguousarray(x.reshape(N, D), dtype=np.float32)
    logits = xf @ gate_w.astype(np.float32) + logit_bias[None, :]      # [N,E]
    full = np.concatenate(
        [logits, np.full((N, NULL), null_logit, np.float32)], axis=1
    )                                                                  # [N,E+NULL]

    # jax.lax.top_k: 4 largest, ties broken by lowest index.
    idx = np.argsort(-full, axis=1, kind="stable")[:, :K]              # [N,K]
    is_null = idx >= E

    # softmax over all E+NULL slots
    m = full.max(axis=1, keepdims=True)
    ex = np.exp(full - m)
    probs = ex / ex.sum(axis=1, keepdims=True)
    topk_w = np.take_along_axis(probs, idx, axis=1)
    real_w = topk_w * (~is_null)
    denom = np.clip(real_w.sum(axis=1, keepdims=True), 1e-6, None)
    w = (real_w / denom).astype(np.float32)                            # [N,K]

    # per-expert token/weight lists
    e_flat = idx.reshape(-1)
    n_flat = np.repeat(np.arange(N), K)
    w_flat = w.reshape(-1)
    valid = e_flat < E
    tok, wt = [], []
    for e in range(E):
        sel = valid & (e_flat == e)
        tok.append(n_flat[sel])
        wt.append(w_flat[sel].astype(np.float32))

    # ---- aux losses (mirrors reference) ----
    ml = logits.max(axis=1, keepdims=True)
    exl = np.exp(logits - ml)
    P_real = (exl / exl.sum(axis=1, keepdims=True)).mean(axis=0)       # [E]
    counts = np.array([len(t) for t in tok], dtype=np.float32)
    f_real = counts / np.clip(counts.sum(), 1e-6, None)
    L_bal = E * np.sum(f_real * P_real)
    null_rate = is_null.astype(np.float32).mean()
    L_null = (null_rate - RHO) ** 2
    lse = (m[:, 0] + np.log(ex.sum(axis=1))).astype(np.float32)
    L_z = (lse.astype(np.float32) ** 2).mean()
    aux = np.float32(0.02 * L_bal + 0.001 * L_z + 0.01 * L_null)

    return xf, tok, wt, aux


# ---------------------------------------------------------------------------
# Bass/Tile device kernel: per core, 3 SwiGLU GEMM groups
#   group = (x^T [D, C], Wg [D,H], Wu [D,H], Wd [H,D]) -> y^T [D, C]
# ---------------------------------------------------------------------------

def _patch_tile_drain():
    """This walrus build rejects >1 embedded sem-wait per instruction.
    Split extra waits onto preceding same-engine NoOps (and extra drains
    for the kernel-tail drain)."""
    import concourse.mybir as mybir
    import concourse.tile as tile_mod
    from concourse.vector_clock import ScopedClock

    if getattr(tile_mod.TileContext, "_drain_split_patched", False):
        return

    MAXW = 1
    _orig_lower = tile_mod.TileContext._lower_ordered_insts

    def _lower_ordered_insts(self, ordered):
        for bb_name, insts in ordered.items():
            out = []
            for inst in insts:
                si = inst.sync_info
                if si is not None and len(si.on_wait) > MAXW:
                    waits = list(si.on_wait)
                    for i, w in enumerate(waits[MAXW:]):
                        nop = mybir.InstNoOp(
                            name=f"{inst.name}_xw{i}",
                            engine=inst.engine,
                            sync_info=mybir.SyncInfo(
                                on_wait=[w], on_update=[]
                            ),
                            bass_nofuse=True,
                        )
                        out.append(nop)
                    inst.sync_info = mybir.SyncInfo(
                        on_wait=waits[:MAXW], on_update=list(si.on_update)
                    )
                out.append(inst)
            ordered[bb_name] = out
        return _orig_lower(self, ordered)

    tile_mod.TileContext._lower_ordered_insts = _lower_ordered_insts

    def _drain_and_barrier(self, tick_clock, wait_clock):
        nc = self.nc
        drain_inst = nc.sync.drain()
        wait_clock.add_sem_waits(
            drain_inst.ins, ScopedClock({None: tick_clock.global_clock})
        )
        si = drain_inst.ins.sync_info
        waits = list(si.on_wait) if si is not None else []
        if len(waits) > 1:
            drain_inst.ins.sync_info = mybir.SyncInfo(
                on_wait=[waits[0]], on_update=[]
            )
            for extra in waits[1:]:
                d2 = nc.sync.drain()
                d2.ins.sync_info = mybir.SyncInfo(on_wait=[extra], on_update=[])
        nc.all_engine_barrier()
        assert self.sems is not None
        popped = nc._tile_sem_poison_stack.pop()
        assert popped is self._sem_poison
        nc.clear_and_free_semaphores(list(self.sems.allocated().values()))
        nc.all_engine_barrier()

    tile_mod.TileContext._drain_and_barrier = _drain_and_barrier
    tile_mod.TileContext._drain_split_patched = True


def _chunk_sizes(total, cap):
    n = math.ceil(total / cap)
    base = total // n
    rem = total - base * n
    return [base + (1 if i < rem else 0) for i in range(n)]


def build_kernel(r_cap, s_cap):
    import concourse.bass as bass
    import concourse.mybir as mybir
    import concourse.tile as tile

    _patch_tile_drain()

    bf = mybir.dt.bfloat16
    f32 = mybir.dt.float32

    nc = bass.Bass(target_bir_lowering=False, debug=False)

    groups = []
    for gname, cap in (("a", r_cap), ("b", r_cap), ("s", s_cap)):
        xt = nc.declare_dram_parameter(f"x{gname}", [D, cap], bf, isOutput=False)
        wg = nc.declare_dram_parameter(f"wg{gname}", [D, H], bf, isOutput=False)
        wu = nc.declare_dram_parameter(f"wu{gname}", [D, H], bf, isOutput=False)
        wd = nc.declare_dram_parameter(f"wd{gname}", [H, D], bf, isOutput=False)
        yt = nc.declare_dram_parameter(f"y{gname}", [D, cap], bf, isOutput=True)
        groups.append((xt, wg, wu, wd, yt, cap))

    KD = D // 128   # 8 contraction tiles for gate/up
    KH = H // 128   # 16 contraction tiles for down
    HT = H // 128   # 16 output tiles of h
    DT = D // 128   # 8 output tiles of y

    with tile.TileContext(nc) as tc:
        with (
            tc.tile_pool(name="wgu", bufs=1) as p_wgu,
            tc.tile_pool(name="wd", bufs=2) as p_wd,
            tc.tile_pool(name="xt", bufs=2) as p_xt,
            tc.tile_pool(name="h", bufs=2) as p_h,
            tc.tile_pool(name="tmp", bufs=3) as p_tmp,
            tc.tile_pool(name="yo", bufs=3) as p_yo,
            tc.tile_pool(name="ps", bufs=2, space="PSUM") as p_ps,
            tc.tile_pool(name="psy", bufs=2, space="PSUM") as p_psy,
        ):
            for xt, wg, wu, wd, yt, cap in groups:
                # resident weights for this group
                wg_t = []
                wu_t = []
                for kd in range(KD):
                    t = p_wgu.tile([128, H], bf, tag=f"wg{kd}")
                    nc.sync.dma_start(out=t[:], in_=wg[kd * 128:(kd + 1) * 128, :])
                    wg_t.append(t)
                    t = p_wgu.tile([128, H], bf, tag=f"wu{kd}")
                    nc.sync.dma_start(out=t[:], in_=wu[kd * 128:(kd + 1) * 128, :])
                    wu_t.append(t)
                wd_t = []
                for kh in range(KH):
                    t = p_wd.tile([128, D], bf, tag=f"wd{kh}")
                    nc.sync.dma_start(out=t[:], in_=wd[kh * 128:(kh + 1) * 128, :])
                    wd_t.append(t)

                c0 = 0
                for cs in _chunk_sizes(cap, 384):
                    csl = slice(c0, c0 + cs)
                    x_t = []
                    for kd in range(KD):
                        t = p_xt.tile([128, cs], bf, tag=f"x{kd}")
                        nc.sync.dma_start(
                            out=t[:], in_=xt[kd * 128:(kd + 1) * 128, csl]
                        )
                        x_t.append(t)

                    h_t = []
                    for ht in range(HT):
                        pg = p_ps.tile([128, cs], f32, tag="pg")
                        pu = p_ps.tile([128, cs], f32, tag="pu")
                        hsl = bass.ts(ht, 128)
                        for kd in range(KD):
                            nc.tensor.matmul(
                                pg[:], wg_t[kd][:, hsl], x_t[kd][:],
                                start=(kd == 0), stop=(kd == KD - 1),
                            )
                        for kd in range(KD):
                            nc.tensor.matmul(
                                pu[:], wu_t[kd][:, hsl], x_t[kd][:],
                                start=(kd == 0), stop=(kd == KD - 1),
                            )
                        tmp = p_tmp.tile([128, cs], f32, tag="tmp")
                        nc.scalar.activation(
                            tmp[:], pg[:], mybir.ActivationFunctionType.Silu
                        )
                        h = p_h.tile([128, cs], bf, tag=f"h{ht}")
                        nc.vector.tensor_mul(h[:], tmp[:], pu[:])
                        h_t.append(h)

                    for dt in range(DT):
                        py = p_psy.tile([128, cs], f32, tag="py")
                        dsl = bass.ts(dt, 128)
                        for kh in range(KH):
                            nc.tensor.matmul(
                                py[:], wd_t[kh][:, dsl], h_t[kh][:],
                                start=(kh == 0), stop=(kh == KH - 1),
                            )
                        yo = p_yo.tile([128, cs], bf, tag="yo")
                        nc.vector.tensor_copy(yo[:], py[:])
                        nc.sync.dma_start(
                            out=yt[dt * 128:(dt + 1) * 128, csl], in_=yo[:]
                        )
                    c0 += cs
    return nc


# ---------------------------------------------------------------------------
# Top-level entry
# ---------------------------------------------------------------------------

def kernel(x, gate_w, logit_bias, null_logit, W_gate, W_up, W_down,
           ws_gate, ws_up, ws_down):
    from concourse.bass_utils import run_bass_kernel_spmd

    x = np.asarray(x)
    xf, tok, wt, aux = _route(
        np.asarray(x, np.float32),
        np.asarray(gate_w, np.float32),
        np.asarray(logit_bias, np.float32),
        np.asarray(null_logit, np.float32),
    )

    counts = np.array([len(t) for t in tok])
    # pair experts large+small for balance; capacity = max count rounded up
    order = np.argsort(-counts, kind="stable")
    pairs = [(int(order[i]), int(order[2 * N_CORES - 1 - i])) for i in range(N_CORES)]
    ncap = int(counts.max())
    n_chunks = max(1, math.ceil(ncap / 384))
    chunk = math.ceil(ncap / n_chunks / 8) * 8
    r_cap = chunk * n_chunks
    s_cap = N // N_CORES

    nc = build_kernel(r_cap, s_cap)

    wbf = {
        "g": np.asarray(W_gate, np.float32).astype(BF16),
        "u": np.asarray(W_up, np.float32).astype(BF16),
        "d": np.asarray(W_down, np.float32).astype(BF16),
    }
    wsg = np.ascontiguousarray(np.asarray(ws_gate, np.float32).astype(BF16))
    wsu = np.ascontiguousarray(np.asarray(ws_up, np.float32).astype(BF16))
    wsd = np.ascontiguousarray(np.asarray(ws_down, np.float32).astype(BF16))
    xbf = xf.astype(BF16)

    def gathered_xt(e):
        xe = np.zeros((r_cap, D), dtype=BF16)
        xe[: len(tok[e])] = xbf[tok[e]]
        return np.ascontiguousarray(xe.T)

    in_maps = []
    for c, (ea, eb) in enumerate(pairs):
        m = {
            "xa": gathered_xt(ea),
            "xb": gathered_xt(eb),
            "xs": np.ascontiguousarray(xbf[c * s_cap:(c + 1) * s_cap].T),
            "wgs": wsg, "wus": wsu, "wds": wsd,
        }
        for gname, e in (("a", ea), ("b", eb)):
            m[f"wg{gname}"] = np.ascontiguousarray(wbf["g"][e])
            m[f"wu{gname}"] = np.ascontiguousarray(wbf["u"][e])
            m[f"wd{gname}"] = np.ascontiguousarray(wbf["d"][e])
        in_maps.append(m)

    res = run_bass_kernel_spmd(nc, in_maps, list(range(N_CORES)))

    out = np.zeros((N, D), np.float32)
    for c, (ea, eb) in enumerate(pairs):
        for gname, e in (("a", ea), ("b", eb)):
            ye = np.asarray(res.results[c][f"y{gname}"]).astype(np.float32)
            n_e = len(tok[e])
            out[tok[e]] += wt[e][:, None] * ye[:, :n_e].T
        ys = np.asarray(res.results[c]["ys"]).astype(np.float32)
        out[c * s_cap:(c + 1) * s_cap] += ys.T

    return out.reshape(B, T, D), aux


# revision 6
# speedup vs baseline: 1.3203x; 1.3203x over previous
"""MoE FFN (16 experts, top-4, null-expert router) on 8 Trainium2 NeuronCores.

Strategy:
  - Router, top-k selection, combine weights and aux losses are computed on
    host in numpy (~0.15 GFLOP total; the device work is ~258 GFLOP).
  - Routed experts are expert-parallel: 2 experts per core (paired
    large+small count for balance), each expert's gathered tokens padded to
    a common capacity so all 8 cores run an identical program.
  - The shared expert is token-parallel: 512 tokens per core.
  - All GEMMs run on the tensor engine in bf16 with fp32 PSUM accumulation.
  - Host applies combine weights and scatter-adds expert outputs.
"""

import math

import ml_dtypes
import numpy as np

B, T, D, H = 2, 2048, 1024, 2048
E = 16
K = 4
RHO = 0.5
NULL = int(E * (1 - RHO) / RHO)
N = B * T
N_CORES = 8
BF16 = ml_dtypes.bfloat16


# ---------------------------------------------------------------------------
# Host-side router (exact replication of the reference math in numpy)
# ---------------------------------------------------------------------------

def _route(x, gate_w, logit_bias, null_logit):
    xf = np.ascontiguousarray(x.reshape(N, D), dtype=np.float32)
    logits = xf @ gate_w.astype(np.float32) + logit_bias[None, :]      # [N,E]
    full = np.concatenate(
        [logits, np.full((N, NULL), null_logit, np.float32)], axis=1
    )                                                                  # [N,E+NULL]

    # jax.lax.top_k: 4 largest, ties broken by lowest index.
    idx = np.argsort(-full, axis=1, kind="stable")[:, :K]              # [N,K]
    is_null = idx >= E

    # softmax over all E+NULL slots
    m = full.max(axis=1, keepdims=True)
    ex = np.exp(full - m)
    probs = ex / ex.sum(axis=1, keepdims=True)
    topk_w = np.take_along_axis(probs, idx, axis=1)
    real_w = topk_w * (~is_null)
    denom = np.clip(real_w.sum(axis=1, keepdims=True), 1e-6, None)
    w = (real_w / denom).astype(np.float32)                            # [N,K]

    # per-expert token/weight lists
    e_flat = idx.reshape(-1)
    n_flat = np.repeat(np.arange(N), K)
    w_flat = w.reshape(-1)
    valid = e_flat < E
    tok, wt = [], []
    for e in range(E):
        sel = valid & (e_flat == e)
        tok.append(n_flat[sel])
        wt.append(w_flat[sel].astype(np.float32))

    # ---- aux losses (mirrors reference) ----
    ml = logits.max(axis=1, keepdims=True)
    exl = np.exp(logits - ml)
    P_real = (exl / exl.sum(axis=1, keepdims=True)).mean(axis=0)       # [E]
    counts = np.array([len(t) for t in tok], dtype=np.float32)
    f_real = counts / np.clip(counts.sum(), 1e-6, None)
    L_bal = E * np.sum(f_real * P_real)
    null_rate = is_null.astype(np.float32).mean()
    L_null = (null_rate - RHO) ** 2
    lse = (m[:, 0] + np.log(ex.sum(axis=1))).astype(np.float32)
    L_z = (lse.astype(np.float32) ** 2).mean()
    aux = np.float32(0.02 * L_bal + 0.001 * L_z + 0.01 * L_null)

    return xf, tok, wt, aux


# ---------------------------------------------------------------------------
# Bass/Tile device kernel: per core, 3 SwiGLU GEMM groups
#   group = (x^T [D, C], Wg [D,H], Wu [D,H], Wd [H,D]) -> y^T [D, C]
# ---------------------------------------------------------------------------

def _patch_tile_drain():
    """This walrus build rejects >1 embedded sem-wait per instruction.
    Split extra waits onto preceding same-engine NoOps (and extra drains
    for the kernel-tail drain)."""
    import concourse.mybir as mybir
    import concourse.tile as tile_mod
    from concourse.vector_clock import ScopedClock

    if getattr(tile_mod.TileContext, "_drain_split_patched", False):
        return

    MAXW = 1
    _orig_lower = tile_mod.TileContext._lower_ordered_insts

    def _lower_ordered_insts(self, ordered):
        for bb_name, insts in ordered.items():
            out = []
            for inst in insts:
                si = inst.sync_info
                if si is not None and len(si.on_wait) > MAXW:
                    waits = list(si.on_wait)
                    for i, w in enumerate(waits[MAXW:]):
                        nop = mybir.InstNoOp(
                            name=f"{inst.name}_xw{i}",
                            engine=inst.engine,
                            sync_info=mybir.SyncInfo(
                                on_wait=[w], on_update=[]
                            ),
                            bass_nofuse=True,
                        )
                        out.append(nop)
                    inst.sync_info = mybir.SyncInfo(
                        on_wait=waits[:MAXW], on_update=list(si.on_update)
                    )
                out.append(inst)
            ordered[bb_name] = out
        return _orig_lower(self, ordered)

    tile_mod.TileContext._lower_ordered_insts = _lower_ordered_insts

    def _drain_and_barrier(self, tick_clock, wait_clock):
        nc = self.nc
        drain_inst = nc.sync.drain()
        wait_clock.add_sem_waits(
            drain_inst.ins, ScopedClock({None: tick_clock.global_clock})
        )
        si = drain_inst.ins.sync_info
        waits = list(si.on_wait) if si is not None else []
        if len(waits) > 1:
            drain_inst.ins.sync_info = mybir.SyncInfo(
                on_wait=[waits[0]], on_update=[]
            )
            for extra in waits[1:]:
                d2 = nc.sync.drain()
                d2.ins.sync_info = mybir.SyncInfo(on_wait=[extra], on_update=[])
        nc.all_engine_barrier()
        assert self.sems is not None
        popped = nc._tile_sem_poison_stack.pop()
        assert popped is self._sem_poison
        nc.clear_and_free_semaphores(list(self.sems.allocated().values()))
        nc.all_engine_barrier()

    tile_mod.TileContext._drain_and_barrier = _drain_and_barrier
    tile_mod.TileContext._drain_split_patched = True


def _chunk_sizes(total, cap):
    n = math.ceil(total / cap)
    base = total // n
    rem = total - base * n
    return [base + (1 if i < rem else 0) for i in range(n)]


def build_kernel(r_cap, s_cap, opt=None):
    import concourse.bass as bass
    import concourse.mybir as mybir
    import concourse.tile as tile

    _patch_tile_drain()

    opt = opt or {}
    cs_cap = opt.get("cs_cap", 384)
    ps_bufs = opt.get("ps_bufs", 2)
    psy_bufs = opt.get("psy_bufs", 2)
    wd_bufs = opt.get("wd_bufs", 2)
    xt_bufs = opt.get("xt_bufs", 2)
    h_bufs = opt.get("h_bufs", 2)
    s_chunks = opt.get("s_chunks", 2)

    bf = mybir.dt.bfloat16
    f32 = mybir.dt.float32

    nc = bass.Bass(target_bir_lowering=False, debug=False)

    groups = []
    for gname, cap in (("a", r_cap), ("b", r_cap), ("s", s_cap)):
        xt = nc.declare_dram_parameter(f"x{gname}", [D, cap], bf, isOutput=False)
        wg = nc.declare_dram_parameter(f"wg{gname}", [D, H], bf, isOutput=False)
        wu = nc.declare_dram_parameter(f"wu{gname}", [D, H], bf, isOutput=False)
        wd = nc.declare_dram_parameter(f"wd{gname}", [H, D], bf, isOutput=False)
        yt = nc.declare_dram_parameter(f"y{gname}", [D, cap], bf, isOutput=True)
        groups.append((xt, wg, wu, wd, yt, cap))

    KD = D // 128   # 8 contraction tiles for gate/up
    KH = H // 128   # 16 contraction tiles for down
    HT = H // 128   # 16 output tiles of h
    DT = D // 128   # 8 output tiles of y

    with tile.TileContext(nc) as tc:
        with (
            tc.tile_pool(name="wgu", bufs=1) as p_wgu,
            tc.tile_pool(name="wd", bufs=wd_bufs) as p_wd,
            tc.tile_pool(name="xt", bufs=xt_bufs) as p_xt,
            tc.tile_pool(name="h", bufs=h_bufs) as p_h,
            tc.tile_pool(name="tmp", bufs=3) as p_tmp,
            tc.tile_pool(name="yo", bufs=3) as p_yo,
            tc.tile_pool(name="ps", bufs=ps_bufs, space="PSUM") as p_ps,
            tc.tile_pool(name="psy", bufs=psy_bufs, space="PSUM") as p_psy,
        ):
            for xt, wg, wu, wd, yt, cap in groups:
                n_ch = s_chunks if cap == s_cap and s_cap != r_cap else None
                sizes = (_chunk_sizes(cap, cs_cap) if n_ch is None
                         else _chunk_sizes(cap, math.ceil(cap / n_ch)))

                # first chunk's activations land before the weights so the
                # gate matmuls can start as early as possible
                x0_t = []
                for kd in range(KD):
                    t = p_xt.tile([128, sizes[0]], bf, tag=f"x{kd}")
                    nc.sync.dma_start(out=t[:], in_=xt[kd * 128:(kd + 1) * 128,
                                                       0:sizes[0]])
                    x0_t.append(t)

                # resident weights for this group
                wg_t = []
                wu_t = []
                for kd in range(KD):
                    t = p_wgu.tile([128, H], bf, tag=f"wg{kd}")
                    nc.sync.dma_start(out=t[:], in_=wg[kd * 128:(kd + 1) * 128, :])
                    wg_t.append(t)
                for kd in range(KD):
                    t = p_wgu.tile([128, H], bf, tag=f"wu{kd}")
                    nc.sync.dma_start(out=t[:], in_=wu[kd * 128:(kd + 1) * 128, :])
                    wu_t.append(t)
                wd_t = []
                for kh in range(KH):
                    t = p_wd.tile([128, D], bf, tag=f"wd{kh}")
                    nc.sync.dma_start(out=t[:], in_=wd[kh * 128:(kh + 1) * 128, :])
                    wd_t.append(t)

                c0 = 0
                for ci, cs in enumerate(sizes):
                    csl = slice(c0, c0 + cs)
                    if ci == 0:
                        x_t = x0_t
                    else:
                        x_t = []
                        for kd in range(KD):
                            t = p_xt.tile([128, cs], bf, tag=f"x{kd}")
                            nc.sync.dma_start(
                                out=t[:], in_=xt[kd * 128:(kd + 1) * 128, csl]
                            )
                            x_t.append(t)

                    h_t = []
                    for ht in range(HT):
                        pg = p_ps.tile([128, cs], f32, tag="pg")
                        pu = p_ps.tile([128, cs], f32, tag="pu")
                        hsl = bass.ts(ht, 128)
                        for kd in range(KD):
                            nc.tensor.matmul(
                                pg[:], wg_t[kd][:, hsl], x_t[kd][:],
                                start=(kd == 0), stop=(kd == KD - 1),
                            )
                        for kd in range(KD):
                            nc.tensor.matmul(
                                pu[:], wu_t[kd][:, hsl], x_t[kd][:],
                                start=(kd == 0), stop=(kd == KD - 1),
                            )
                        tmp = p_tmp.tile([128, cs], f32, tag="tmp")
                        nc.scalar.activation(
                            tmp[:], pg[:], mybir.ActivationFunctionType.Silu
                        )
                        h = p_h.tile([128, cs], bf, tag=f"h{ht}")
                        nc.vector.tensor_mul(h[:], tmp[:], pu[:])
                        h_t.append(h)

                    for dt in range(DT):
                        py = p_psy.tile([128, cs], f32, tag="py")
                        dsl = bass.ts(dt, 128)
                        for kh in range(KH):
                            nc.tensor.matmul(
                                py[:], wd_t[kh][:, dsl], h_t[kh][:],
                                start=(kh == 0), stop=(kh == KH - 1),
                            )
                        yo = p_yo.tile([128, cs], bf, tag="yo")
                        nc.vector.tensor_copy(yo[:], py[:])
                        nc.sync.dma_start(
                            out=yt[dt * 128:(dt + 1) * 128, csl], in_=yo[:]
                        )
                    c0 += cs
    return nc


# ---------------------------------------------------------------------------
# Top-level entry
# ---------------------------------------------------------------------------

def kernel(x, gate_w, logit_bias, null_logit, W_gate, W_up, W_down,
           ws_gate, ws_up, ws_down):
    from concourse.bass_utils import run_bass_kernel_spmd

    x = np.asarray(x)
    xf, tok, wt, aux = _route(
        np.asarray(x, np.float32),
        np.asarray(gate_w, np.float32),
        np.asarray(logit_bias, np.float32),
        np.asarray(null_logit, np.float32),
    )

    counts = np.array([len(t) for t in tok])
    # pair experts large+small for balance; capacity = max count rounded up
    order = np.argsort(-counts, kind="stable")
    pairs = [(int(order[i]), int(order[2 * N_CORES - 1 - i])) for i in range(N_CORES)]
    ncap = int(counts.max())
    n_chunks = max(1, math.ceil(ncap / 384))
    chunk = math.ceil(ncap / n_chunks / 8) * 8
    r_cap = chunk * n_chunks
    s_cap = N // N_CORES

    nc = build_kernel(r_cap, s_cap)

    wbf = {
        "g": np.asarray(W_gate, np.float32).astype(BF16),
        "u": np.asarray(W_up, np.float32).astype(BF16),
        "d": np.asarray(W_down, np.float32).astype(BF16),
    }
    wsg = np.ascontiguousarray(np.asarray(ws_gate, np.float32).astype(BF16))
    wsu = np.ascontiguousarray(np.asarray(ws_up, np.float32).astype(BF16))
    wsd = np.ascontiguousarray(np.asarray(ws_down, np.float32).astype(BF16))
    xbf = xf.astype(BF16)

    def gathered_xt(e):
        xe = np.zeros((r_cap, D), dtype=BF16)
        xe[: len(tok[e])] = xbf[tok[e]]
        return np.ascontiguousarray(xe.T)

    in_maps = []
    for c, (ea, eb) in enumerate(pairs):
        m = {
            "xa": gathered_xt(ea),
            "xb": gathered_xt(eb),
            "xs": np.ascontiguousarray(xbf[c * s_cap:(c + 1) * s_cap].T),
            "wgs": wsg, "wus": wsu, "wds": wsd,
        }
        for gname, e in (("a", ea), ("b", eb)):
            m[f"wg{gname}"] = np.ascontiguousarray(wbf["g"][e])
            m[f"wu{gname}"] = np.ascontiguousarray(wbf["u"][e])
            m[f"wd{gname}"] = np.ascontiguousarray(wbf["d"][e])
        in_maps.append(m)

    res = run_bass_kernel_spmd(nc, in_maps, list(range(N_CORES)))

    out = np.zeros((N, D), np.float32)
    for c, (ea, eb) in enumerate(pairs):
        for gname, e in (("a", ea), ("b", eb)):
            ye = np.asarray(res.results[c][f"y{gname}"]).astype(np.float32)
            n_e = len(tok[e])
            out[tok[e]] += wt[e][:, None] * ye[:, :n_e].T
        ys = np.asarray(res.results[c]["ys"]).astype(np.float32)
        out[c * s_cap:(c + 1) * s_cap] += ys.T

    return out.reshape(B, T, D), aux


# revision 8
# speedup vs baseline: 3.3602x; 2.5451x over previous
"""MoE FFN (16 experts, top-4, null-expert router) on 8 Trainium2 NeuronCores.

Strategy:
  - Router, top-k selection, combine weights and aux losses are computed on
    host in numpy (~0.15 GFLOP total; the device work is ~258 GFLOP).
  - Routed experts are expert-parallel: 2 experts per core (paired
    large+small count for balance), each expert's gathered tokens padded to
    a common capacity so all 8 cores run an identical program.
  - The shared expert is token-parallel: 512 tokens per core.
  - All GEMMs run on the tensor engine in bf16 with fp32 PSUM accumulation.
  - Host applies combine weights and scatter-adds expert outputs.
"""

import math

import ml_dtypes
import numpy as np

B, T, D, H = 2, 2048, 1024, 2048
E = 16
K = 4
RHO = 0.5
NULL = int(E * (1 - RHO) / RHO)
N = B * T
N_CORES = 8
BF16 = ml_dtypes.bfloat16


# ---------------------------------------------------------------------------
# Host-side router (exact replication of the reference math in numpy)
# ---------------------------------------------------------------------------

def _route(x, gate_w, logit_bias, null_logit):
    xf = np.ascontiguousarray(x.reshape(N, D), dtype=np.float32)
    logits = xf @ gate_w.astype(np.float32) + logit_bias[None, :]      # [N,E]
    full = np.concatenate(
        [logits, np.full((N, NULL), null_logit, np.float32)], axis=1
    )                                                                  # [N,E+NULL]

    # jax.lax.top_k: 4 largest, ties broken by lowest index.
    idx = np.argsort(-full, axis=1, kind="stable")[:, :K]              # [N,K]
    is_null = idx >= E

    # softmax over all E+NULL slots
    m = full.max(axis=1, keepdims=True)
    ex = np.exp(full - m)
    probs = ex / ex.sum(axis=1, keepdims=True)
    topk_w = np.take_along_axis(probs, idx, axis=1)
    real_w = topk_w * (~is_null)
    denom = np.clip(real_w.sum(axis=1, keepdims=True), 1e-6, None)
    w = (real_w / denom).astype(np.float32)                            # [N,K]

    # per-expert token/weight lists
    e_flat = idx.reshape(-1)
    n_flat = np.repeat(np.arange(N), K)
    w_flat = w.reshape(-1)
    valid = e_flat < E
    tok, wt = [], []
    for e in range(E):
        sel = valid & (e_flat == e)
        tok.append(n_flat[sel])
        wt.append(w_flat[sel].astype(np.float32))

    # ---- aux losses (mirrors reference) ----
    ml = logits.max(axis=1, keepdims=True)
    exl = np.exp(logits - ml)
    P_real = (exl / exl.sum(axis=1, keepdims=True)).mean(axis=0)       # [E]
    counts = np.array([len(t) for t in tok], dtype=np.float32)
    f_real = counts / np.clip(counts.sum(), 1e-6, None)
    L_bal = E * np.sum(f_real * P_real)
    null_rate = is_null.astype(np.float32).mean()
    L_null = (null_rate - RHO) ** 2
    lse = (m[:, 0] + np.log(ex.sum(axis=1))).astype(np.float32)
    L_z = (lse.astype(np.float32) ** 2).mean()
    aux = np.float32(0.02 * L_bal + 0.001 * L_z + 0.01 * L_null)

    return xf, tok, wt, aux


# ---------------------------------------------------------------------------
# Bass/Tile device kernel: per core, 3 SwiGLU GEMM groups
#   group = (x^T [D, C], Wg [D,H], Wu [D,H], Wd [H,D]) -> y^T [D, C]
# ---------------------------------------------------------------------------

def _patch_tile_drain():
    """This walrus build rejects >1 embedded sem-wait per instruction.
    Split extra waits onto preceding same-engine NoOps (and extra drains
    for the kernel-tail drain)."""
    import concourse.mybir as mybir
    import concourse.tile as tile_mod
    from concourse.vector_clock import ScopedClock

    if getattr(tile_mod.TileContext, "_drain_split_patched", False):
        return

    MAXW = 1
    _orig_lower = tile_mod.TileContext._lower_ordered_insts

    def _lower_ordered_insts(self, ordered):
        for bb_name, insts in ordered.items():
            out = []
            for inst in insts:
                si = inst.sync_info
                if si is not None and len(si.on_wait) > MAXW:
                    waits = list(si.on_wait)
                    for i, w in enumerate(waits[MAXW:]):
                        nop = mybir.InstNoOp(
                            name=f"{inst.name}_xw{i}",
                            engine=inst.engine,
                            sync_info=mybir.SyncInfo(
                                on_wait=[w], on_update=[]
                            ),
                            bass_nofuse=True,
                        )
                        out.append(nop)
                    inst.sync_info = mybir.SyncInfo(
                        on_wait=waits[:MAXW], on_update=list(si.on_update)
                    )
                out.append(inst)
            ordered[bb_name] = out
        return _orig_lower(self, ordered)

    tile_mod.TileContext._lower_ordered_insts = _lower_ordered_insts

    def _drain_and_barrier(self, tick_clock, wait_clock):
        nc = self.nc
        drain_inst = nc.sync.drain()
        wait_clock.add_sem_waits(
            drain_inst.ins, ScopedClock({None: tick_clock.global_clock})
        )
        si = drain_inst.ins.sync_info
        waits = list(si.on_wait) if si is not None else []
        if len(waits) > 1:
            drain_inst.ins.sync_info = mybir.SyncInfo(
                on_wait=[waits[0]], on_update=[]
            )
            for extra in waits[1:]:
                d2 = nc.sync.drain()
                d2.ins.sync_info = mybir.SyncInfo(on_wait=[extra], on_update=[])
        nc.all_engine_barrier()
        assert self.sems is not None
        popped = nc._tile_sem_poison_stack.pop()
        assert popped is self._sem_poison
        nc.clear_and_free_semaphores(list(self.sems.allocated().values()))
        nc.all_engine_barrier()

    tile_mod.TileContext._drain_and_barrier = _drain_and_barrier
    tile_mod.TileContext._drain_split_patched = True


def _chunk_sizes(total, cap):
    n = math.ceil(total / cap)
    base = total // n
    rem = total - base * n
    return [base + (1 if i < rem else 0) for i in range(n)]


def build_kernel(r_cap, s_cap, opt=None):
    import concourse.bass as bass
    import concourse.mybir as mybir
    import concourse.tile as tile

    _patch_tile_drain()

    opt = opt or {}
    cs_cap = opt.get("cs_cap", 384)
    ps_bufs = opt.get("ps_bufs", 3)
    psy_bufs = opt.get("psy_bufs", 2)
    wd_bufs = opt.get("wd_bufs", 2)
    xt_bufs = opt.get("xt_bufs", 2)
    h_bufs = opt.get("h_bufs", 2)
    s_chunks = opt.get("s_chunks", 1)

    bf = mybir.dt.bfloat16
    f32 = mybir.dt.float32

    nc = bass.Bass(target_bir_lowering=False, debug=False)

    groups = []
    for gname, cap in (("a", r_cap), ("b", r_cap), ("s", s_cap)):
        xt = nc.declare_dram_parameter(f"x{gname}", [D, cap], bf, isOutput=False)
        wg = nc.declare_dram_parameter(f"wg{gname}", [D, H], bf, isOutput=False)
        wu = nc.declare_dram_parameter(f"wu{gname}", [D, H], bf, isOutput=False)
        wd = nc.declare_dram_parameter(f"wd{gname}", [H, D], bf, isOutput=False)
        yt = nc.declare_dram_parameter(f"y{gname}", [D, cap], bf, isOutput=True)
        groups.append((xt, wg, wu, wd, yt, cap))

    KD = D // 128   # 8 contraction tiles for gate/up
    KH = H // 128   # 16 contraction tiles for down
    HT = H // 128   # 16 output tiles of h
    DT = D // 128   # 8 output tiles of y

    with tile.TileContext(nc) as tc:
        with (
            tc.tile_pool(name="wgu", bufs=1) as p_wgu,
            tc.tile_pool(name="wd", bufs=wd_bufs) as p_wd,
            tc.tile_pool(name="xt", bufs=xt_bufs) as p_xt,
            tc.tile_pool(name="h", bufs=h_bufs) as p_h,
            tc.tile_pool(name="tmp", bufs=3) as p_tmp,
            tc.tile_pool(name="yo", bufs=3) as p_yo,
            tc.tile_pool(name="ps", bufs=ps_bufs, space="PSUM") as p_ps,
            tc.tile_pool(name="psy", bufs=psy_bufs, space="PSUM") as p_psy,
        ):
            for xt, wg, wu, wd, yt, cap in groups:
                n_ch = s_chunks if cap == s_cap and s_cap != r_cap else None
                sizes = (_chunk_sizes(cap, cs_cap) if n_ch is None
                         else _chunk_sizes(cap, math.ceil(cap / n_ch)))

                # first chunk's activations land before the weights so the
                # gate matmuls can start as early as possible
                x0_t = []
                for kd in range(KD):
                    t = p_xt.tile([128, sizes[0]], bf, tag=f"x{kd}")
                    nc.sync.dma_start(out=t[:], in_=xt[kd * 128:(kd + 1) * 128,
                                                       0:sizes[0]])
                    x0_t.append(t)

                # resident weights for this group; loads split into column
                # slices so early h-tiles unblock before the full load lands
                wsplit = opt.get("wsplit", 512)
                wg_t = []
                wu_t = []
                for kd in range(KD):
                    tg = p_wgu.tile([128, H], bf, tag=f"wg{kd}")
                    tu = p_wgu.tile([128, H], bf, tag=f"wu{kd}")
                    wg_t.append(tg)
                    wu_t.append(tu)
                for h0 in range(0, H, wsplit):
                    hs = slice(h0, h0 + wsplit)
                    for kd in range(KD):
                        rows = slice(kd * 128, (kd + 1) * 128)
                        nc.sync.dma_start(out=wg_t[kd][:, hs], in_=wg[rows, hs])
                        nc.sync.dma_start(out=wu_t[kd][:, hs], in_=wu[rows, hs])
                wd_t = []
                for kh in range(KH):
                    t = p_wd.tile([128, D], bf, tag=f"wd{kh}")
                    nc.sync.dma_start(out=t[:], in_=wd[kh * 128:(kh + 1) * 128, :])
                    wd_t.append(t)

                c0 = 0
                for ci, cs in enumerate(sizes):
                    csl = slice(c0, c0 + cs)
                    if ci == 0:
                        x_t = x0_t
                    else:
                        x_t = []
                        for kd in range(KD):
                            t = p_xt.tile([128, cs], bf, tag=f"x{kd}")
                            nc.sync.dma_start(
                                out=t[:], in_=xt[kd * 128:(kd + 1) * 128, csl]
                            )
                            x_t.append(t)

                    h_t = []
                    for ht in range(HT):
                        pg = p_ps.tile([128, cs], f32, tag="pg")
                        pu = p_ps.tile([128, cs], f32, tag="pu")
                        hsl = bass.ts(ht, 128)
                        for kd in range(KD):
                            nc.tensor.matmul(
                                pg[:], wg_t[kd][:, hsl], x_t[kd][:],
                                start=(kd == 0), stop=(kd == KD - 1),
                            )
                        for kd in range(KD):
                            nc.tensor.matmul(
                                pu[:], wu_t[kd][:, hsl], x_t[kd][:],
                                start=(kd == 0), stop=(kd == KD - 1),
                            )
                        tmp = p_tmp.tile([128, cs], f32, tag="tmp")
                        nc.scalar.activation(
                            tmp[:], pg[:], mybir.ActivationFunctionType.Silu
                        )
                        h = p_h.tile([128, cs], bf, tag=f"h{ht}")
                        nc.vector.tensor_mul(h[:], tmp[:], pu[:])
                        h_t.append(h)

                    for dt in range(DT):
                        py = p_psy.tile([128, cs], f32, tag="py")
                        dsl = bass.ts(dt, 128)
                        for kh in range(KH):
                            nc.tensor.matmul(
                                py[:], wd_t[kh][:, dsl], h_t[kh][:],
                                start=(kh == 0), stop=(kh == KH - 1),
                            )
                        yo = p_yo.tile([128, cs], bf, tag="yo")
                        nc.vector.tensor_copy(yo[:], py[:])
                        nc.sync.dma_start(
                            out=yt[dt * 128:(dt + 1) * 128, csl], in_=yo[:]
                        )
                    c0 += cs
    return nc


# ---------------------------------------------------------------------------
# Top-level entry
# ---------------------------------------------------------------------------

def kernel(x, gate_w, logit_bias, null_logit, W_gate, W_up, W_down,
           ws_gate, ws_up, ws_down):
    from concourse.bass_utils import run_bass_kernel_spmd

    x = np.asarray(x)
    xf, tok, wt, aux = _route(
        np.asarray(x, np.float32),
        np.asarray(gate_w, np.float32),
        np.asarray(logit_bias, np.float32),
        np.asarray(null_logit, np.float32),
    )

    counts = np.array([len(t) for t in tok])
    # pair experts large+small for balance; capacity = max count rounded up
    order = np.argsort(-counts, kind="stable")
    pairs = [(int(order[i]), int(order[2 * N_CORES - 1 - i])) for i in range(N_CORES)]
    ncap = int(counts.max())
    n_chunks = max(1, math.ceil(ncap / 384))
    chunk = math.ceil(ncap / n_chunks / 8) * 8
    r_cap = chunk * n_chunks
    s_cap = N // N_CORES

    nc = build_kernel(r_cap, s_cap)

    wbf = {
        "g": np.asarray(W_gate, np.float32).astype(BF16),
        "u": np.asarray(W_up, np.float32).astype(BF16),
        "d": np.asarray(W_down, np.float32).astype(BF16),
    }
    wsg = np.ascontiguousarray(np.asarray(ws_gate, np.float32).astype(BF16))
    wsu = np.ascontiguousarray(np.asarray(ws_up, np.float32).astype(BF16))
    wsd = np.ascontiguousarray(np.asarray(ws_down, np.float32).astype(BF16))
    xbf = xf.astype(BF16)

    def gathered_xt(e):
        xe = np.zeros((r_cap, D), dtype=BF16)
        xe[: len(tok[e])] = xbf[tok[e]]
        return np.ascontiguousarray(xe.T)

    in_maps = []
    for c, (ea, eb) in enumerate(pairs):
        m = {
            "xa": gathered_xt(ea),
            "xb": gathered_xt(eb),
            "xs": np.ascontiguousarray(xbf[c * s_cap:(c + 1) * s_cap].T),
            "wgs": wsg, "wus": wsu, "wds": wsd,
        }
        for gname, e in (("a", ea), ("b", eb)):
            m[f"wg{gname}"] = np.ascontiguousarray(wbf["g"][e])
            m[f"wu{gname}"] = np.ascontiguousarray(wbf["u"][e])
            m[f"wd{gname}"] = np.ascontiguousarray(wbf["d"][e])
        in_maps.append(m)

    res = run_bass_kernel_spmd(nc, in_maps, list(range(N_CORES)))

    out = np.zeros((N, D), np.float32)
    for c, (ea, eb) in enumerate(pairs):
        for gname, e in (("a", ea), ("b", eb)):
            ye = np.asarray(res.results[c][f"y{gname}"]).astype(np.float32)
            n_e = len(tok[e])
            out[tok[e]] += wt[e][:, None] * ye[:, :n_e].T
        ys = np.asarray(res.results[c]["ys"]).astype(np.float32)
        out[c * s_cap:(c + 1) * s_cap] += ys.T

    return out.reshape(B, T, D), aux
